# revision 1
# baseline (speedup 1.0000x reference)
"""Trainium2 Bass kernel for nn_CausalSelfAttention_38216619000057.

Reference semantics (faithful to the source bug q = k):
    qkv = x @ W_attn + b_attn ; _, k, v = split(qkv)
    S = (K K^T) * D**-0.5  (per head, causal-masked), P = softmax(S)
    out = (P V) reshaped @ W_proj + b_proj

Sharding over 8 cores: data-parallel on B (4), tensor-parallel on heads (2
groups of 8). Core c handles batch c//2, heads 8*(c%2)..8*(c%2)+7, and
produces a partial projection output; the host sums the two partials per
batch and adds b_proj + b_v @ W_proj (the V-bias contribution commutes
through softmax because rows of P sum to 1).

Since q = k, S is symmetric, so S^T tiles (keys on partitions, queries on
the free axis) are computed directly from the same K^T operand, which is
what the P V matmul needs as its moving operand -- no on-chip transposes of
the attention matrix. The causal mask is applied with an extra accumulating
matmul (a clamped ramp -1000*(j-i)^+ built from triangular 0/1 factors)
so exp() flushes masked entries to zero. A column of ones appended to V
makes the same accumulation produce the softmax denominators for free.

All matmul operands are float32r (TF32-like fast mode, ~1e-3 per-dot
error). The softmax is near-identity here (the q=k bug makes the diagonal
dominate), so S-path rounding cancels in E_ii/l and the end-to-end error
stays ~2e-4.

Heads within a pair are packed on partitions 0:64 / 64:128 of the same
K^T tile and their S^T matmuls are issued interleaved so the PE's row-group
concurrency can overlap the two 64-contraction matmuls.
"""

import threading

import numpy as np

import concourse.bacc as bacc
import concourse.mybir as mybir
import concourse.tile as tile
from concourse.bass_utils import run_bass_kernel_spmd
from concourse.masks import make_identity

B, T, D = 4, 2048, 1024
H = 16
HD = 64
NCORES = 8
HPC = 8  # heads per core
ISQ = float(D**-0.5) ** 0.5  # K is pre-scaled by sqrt(D**-0.5)
F32 = mybir.dt.float32
F32R = mybir.dt.float32r
BF16 = mybir.dt.bfloat16
NEG = -1000.0

Ident = mybir.ActivationFunctionType.Identity
Exp = mybir.ActivationFunctionType.Exp
Mult = mybir.AluOpType.mult

_cache_lock = threading.Lock()
_cached_nc = {}


def _declare_io(nc, synth=False):
    kind = "Internal" if synth else "ExternalInput"
    ts = {}
    ts["x"] = nc.dram_tensor("x", [T, D], F32, kind=kind)
    ts["wk"] = nc.dram_tensor("wk", [128, 4, 8, 128], F32R, kind=kind)
    ts["wv"] = nc.dram_tensor("wv", [128, 8, 512], F32R, kind=kind)
    ts["wp"] = nc.dram_tensor("wp", [128, 4, 1024], F32R, kind=kind)
    ts["bk"] = nc.dram_tensor("bk", [128, 4], F32, kind=kind)
    ts["a_neg"] = nc.dram_tensor("a_neg", [128, 128], BF16, kind=kind)
    ts["b_o"] = nc.dram_tensor("b_o", [128, 4, 512], BF16, kind=kind)
    ts["out"] = nc.dram_tensor("out", [T, D], F32, kind="Internal" if synth else "ExternalOutput")
    if synth:
        ts["done"] = nc.dram_tensor("done", [1, 4], F32, kind="ExternalOutput")
    return ts


def _synth_init(nc, tc, io):
    """Fill the Internal input tensors with benign constants on device."""
    with tc.tile_pool(name="init", bufs=1) as pool:
        it = pool.tile([128, 4096], F32, name="init_t")
        nc.vector.memset(it[:], 0.01)
        for tb in range(16):
            nc.sync.dma_start(io["x"][tb * 128 : (tb + 1) * 128, :], it[:, 0:1024])
        nc.sync.dma_start(io["wk"][:], it[:, 0 : 4 * 8 * 128].bitcast(F32R).rearrange("p (a b c) -> p a b c", a=4, b=8))
        nc.sync.dma_start(io["wv"][:], it[:, 0 : 8 * 512].bitcast(F32R).rearrange("p (a b) -> p a b", a=8))
        nc.sync.dma_start(io["wp"][:], it[:, 0 : 4 * 1024].bitcast(F32R).rearrange("p (a b) -> p a b", a=4))
        nc.sync.dma_start(io["bk"][:], it[:, 0:4])
        itb = pool.tile([128, 2048], BF16, name="init_tb")
        nc.vector.memset(itb[:].bitcast(F32)[:, 0:1024], 0.0)
        nc.sync.dma_start(io["a_neg"][:], itb[:, 0:128])
        nc.sync.dma_start(io["b_o"][:], itb[:].rearrange("p (a b) -> p a b", a=4))


def _emit_body(nc, tc, io, g):
    """One full forward pass. g holds the persistent SBUF tiles."""
    kt_sb, v_ones = g["kt_sb"], g["v_ones"]
    wp_sb, an_sb, bo_sb, bk_sb = g["wp_sb"], g["an_sb"], g["bo_sb"], g["bk_sb"]
    ident, ones_sb = g["ident"], g["ones_sb"]
    x, out = io["x"], io["out"]

    # ---------- Phase A: x^T (PE transpose), K^T, V ----------
    with (
        tc.tile_pool(name="wkv", bufs=1) as wkv,
        tc.tile_pool(name="xload", bufs=2) as xload,
        tc.tile_pool(name="xtp", bufs=2) as xtp,
        tc.tile_pool(name="psT", bufs=3, space="PSUM") as psT,
        tc.tile_pool(name="psKV", bufs=4, space="PSUM") as psKV,
    ):
        wk_sb = wkv.tile([128, 4, 8, 128], F32R)
        wv_sb = wkv.tile([128, 8, 512], F32R)
        nc.sync.dma_start(wk_sb[:], io["wk"][:])
        nc.sync.dma_start(wv_sb[:], io["wv"][:])

        for tci in range(4):  # t-chunks of 512
            xt_chunk = xtp.tile([128, 8, 512], F32R, tag="xtc")
            for tbl in range(4):
                xt_ = xload.tile([128, D], F32, tag="xl")
                tb = 4 * tci + tbl
                nc.sync.dma_start(xt_[:], x[tb * 128 : (tb + 1) * 128, :])
                for gg in (0, 1):  # groups of 4 e-blocks
                    tps = psT.tile([128, 512], F32, tag="tp")
                    for ebl in range(4):
                        eb = 4 * gg + ebl
                        nc.tensor.transpose(
                            tps[:, ebl * 128 : (ebl + 1) * 128],
                            xt_[:, eb * 128 : (eb + 1) * 128],
                            ident[:],
                        )
                    nc.vector.tensor_copy(
                        xt_chunk[:, 4 * gg : 4 * gg + 4, tbl * 128 : (tbl + 1) * 128],
                        tps[:].rearrange("p (e c) -> p e c", c=128),
                    )
            # K^T for this t-chunk
            for hp in range(4):
                kps = psKV.tile([128, 512], F32, tag="kv")
                for eb in range(8):
                    nc.tensor.matmul(
                        kps[:],
                        wk_sb[:, hp, eb, :],
                        xt_chunk[:, eb, :],
                        start=(eb == 0),
                        stop=(eb == 7),
                    )
                nc.scalar.activation(
                    kt_sb[:, hp, tci * 512 : (tci + 1) * 512],
                    kps[:],
                    Ident,
                    bias=bk_sb[:, hp : hp + 1],
                    scale=ISQ,
                )
            # V rows for this t-chunk
            for tbl in range(4):
                vps = psKV.tile([128, 512], F32, tag="kv")
                for eb in range(8):
                    nc.tensor.matmul(
                        vps[:],
                        xt_chunk[:, eb, tbl * 128 : (tbl + 1) * 128],
                        wv_sb[:, eb, :],
                        start=(eb == 0),
                        stop=(eb == 7),
                    )
                tb = 4 * tci + tbl
                nc.vector.tensor_copy(
                    v_ones[:, tb, :].rearrange("p (h c) -> p h c", c=65)[:, :, 0:64],
                    vps[:].rearrange("p (h c) -> p h c", c=64),
                )

    # ---------- Phase B: attention, with per-chunk fused projection ----------
    with tc.tile_pool(name="obig", bufs=1) as obig:
        o_t = obig.tile([128, 4, T], F32R, name="o_t")
        with (
            tc.tile_pool(name="ps_s", bufs=3, space="PSUM") as ps_s,
            tc.tile_pool(name="ps_pv", bufs=2, space="PSUM") as ps_pv,
            tc.tile_pool(name="ebuf", bufs=6) as ebuf,
            tc.tile_pool(name="rbuf", bufs=6) as rbuf,
            tc.tile_pool(name="obuf", bufs=3) as obuf,
        ):
            for ci in range(4):
                njb = 4 * ci + 4
                for hp in range(4):
                    rhs = [
                        kt_sb[64 * q : 64 * q + 64, hp, ci * 512 : (ci + 1) * 512]
                        for q in (0, 1)
                    ]
                    pv = [
                        ps_pv.tile([65, 512], F32, tag="pv", name="pv") for _ in (0, 1)
                    ]
                    for jbp in range(njb // 2):
                        sps = [
                            ps_s.tile([128, 1024], F32, tag="s", name="sps")
                            for _ in (0, 1)
                        ]
                        for half in (0, 1):
                            jb = 2 * jbp + half
                            hs = slice(half * 512, half * 512 + 512)
                            crossing = jb >= 4 * ci
                            for q in (0, 1):  # adjacent MMs hit distinct row groups
                                nc.tensor.matmul(
                                    sps[q][:, hs],
                                    kt_sb[
                                        64 * q : 64 * q + 64,
                                        hp,
                                        jb * 128 : (jb + 1) * 128,
                                    ],
                                    rhs[q],
                                    start=True,
                                    stop=not crossing,
                                )
                            if crossing:
                                oi = jb - 4 * ci
                                for q in (0, 1):
                                    nc.tensor.matmul(
                                        sps[q][:, hs],
                                        an_sb[:],
                                        bo_sb[:, oi, :],
                                        start=False,
                                        stop=True,
                                    )
                        eps = []
                        for q in (0, 1):
                            ep = ebuf.tile([128, 1024], F32R, tag="e")
                            nc.scalar.activation(ep[:], sps[q][:], Exp)
                            eps.append(ep)
                        for half in (0, 1):
                            jb = 2 * jbp + half
                            hs = slice(half * 512, half * 512 + 512)
                            for q in (0, 1):
                                hl = 2 * hp + q
                                nc.tensor.matmul(
                                    pv[q][:],
                                    v_ones[:, jb, 65 * hl : 65 * hl + 65],
                                    eps[q][:, hs],
                                    start=(jb == 0),
                                    stop=(jb == njb - 1),
                                )
                    for q in (0, 1):
                        r_row = rbuf.tile([1, 512], F32R, tag="rr")
                        with nc.allow_low_precision(
                            reason="f32r reciprocal output feeds bc matmul"
                        ):
                            nc.vector.reciprocal(r_row[:], pv[q][64:65, :])
                        bcps = ps_s.tile([64, 512], F32, tag="s", name="bcps")
                        nc.tensor.matmul(
                            bcps[:], ones_sb[:], r_row[:], start=True, stop=True
                        )
                        r_bc = rbuf.tile([64, 512], F32, tag="rb")
                        nc.vector.tensor_copy(r_bc[:], bcps[:])
                        nc.vector.tensor_tensor(
                            o_t[64 * q : 64 * q + 64, hp, ci * 512 : (ci + 1) * 512],
                            pv[q][0:64, :],
                            r_bc[:],
                            Mult,
                        )
                # fused output projection for this chunk's 4 t-blocks
                for tbl in range(4):
                    tb = 4 * ci + tbl
                    for nch in range(2):
                        ops_ = ps_s.tile([128, 512], F32, tag="s", name="ops")
                        for hp2 in range(4):
                            nc.tensor.matmul(
                                ops_[:],
                                o_t[:, hp2, tb * 128 : (tb + 1) * 128],
                                wp_sb[:, hp2, nch * 512 : (nch + 1) * 512],
                                start=(hp2 == 0),
                                stop=(hp2 == 3),
                            )
                        ob = obuf.tile([128, 512], F32, tag="ob")
                        nc.vector.tensor_copy(ob[:], ops_[:])
                        nc.sync.dma_start(
                            out[
                                tb * 128 : (tb + 1) * 128, nch * 512 : (nch + 1) * 512
                            ],
                            ob[:],
                        )

def _build_program(nreps: int = 1, synth: bool = False):
    nc = bacc.Bacc("TRN2", target_bir_lowering=False)
    io = _declare_io(nc, synth=synth)

    with tile.TileContext(nc) as tc:
        if synth:
            _synth_init(nc, tc, io)
        with tc.tile_pool(name="singles", bufs=1) as singles:
            g = {}
            g["kt_sb"] = singles.tile([128, 4, T], F32R, name="kt_sb")
            g["v_ones"] = singles.tile([128, 16, HPC * 65], F32R, name="v_ones")
            g["wp_sb"] = singles.tile([128, 4, 1024], F32R, name="wp_sb")
            g["an_sb"] = singles.tile([128, 128], BF16, name="an_sb")
            g["bo_sb"] = singles.tile([128, 4, 512], BF16, name="bo_sb")
            g["bk_sb"] = singles.tile([128, 4], F32, name="bk_sb")
            g["ident"] = singles.tile([128, 128], F32, name="ident")
            g["ones_sb"] = singles.tile([1, 64], F32R, name="ones_sb")

            nc.sync.dma_start(g["wp_sb"][:], io["wp"][:])
            nc.sync.dma_start(g["an_sb"][:], io["a_neg"][:])
            nc.sync.dma_start(g["bo_sb"][:], io["b_o"][:])
            nc.sync.dma_start(g["bk_sb"][:], io["bk"][:])
            make_identity(nc, g["ident"][:])
            nc.vector.memset(g["ones_sb"][:].bitcast(F32), 1.0)
            nc.vector.memset(
                g["v_ones"][:]
                .bitcast(F32)
                .rearrange("p t (h c) -> p t h c", c=65)[:, :, :, 64:65],
                1.0,
            )

            for _rep in range(nreps):
                _emit_body(nc, tc, io, g)

            if synth:
                with tc.tile_pool(name="fin", bufs=1) as fin:
                    dn = fin.tile([1, 4], F32, name="dn")
                    nc.vector.memset(dn[:], 1.0)
                    nc.sync.dma_start(io["done"][:], dn[:])

    nc.compile()
    return nc


def _build_null_program():
    """Same I/O signature, trivial body -- for wall-clock differencing."""
    nc = bacc.Bacc("TRN2", target_bir_lowering=False)
    io = _declare_io(nc)
    with tile.TileContext(nc) as tc:
        with tc.tile_pool(name="sb", bufs=2) as sb:
            t = sb.tile([128, 512], F32)
            nc.sync.dma_start(t[:], io["x"][0:128, 0:512])
            for tb in range(16):
                for nch in range(2):
                    nc.sync.dma_start(
                        io["out"][
                            tb * 128 : (tb + 1) * 128, nch * 512 : (nch + 1) * 512
                        ],
                        t[:],
                    )
    nc.compile()
    return nc


def _get_program(nreps: int = 1, synth: bool = False):
    with _cache_lock:
        key = (nreps, synth)
        if key not in _cached_nc:
            _cached_nc[key] = _build_program(nreps, synth)
        return _cached_nc[key]


def _core_inputs(c, x, W_attn, b_attn, a_np, b_np):
    b = c // 2
    h0 = HPC * (c % 2)
    c0k = D + h0 * HD
    c0v = 2 * D + h0 * HD
    wk_np = np.ascontiguousarray(
        W_attn[:, c0k : c0k + 512].reshape(8, 128, 4, 128).transpose(1, 2, 0, 3)
    )
    wv_np = np.ascontiguousarray(
        W_attn[:, c0v : c0v + 512].reshape(8, 128, 512).transpose(1, 0, 2)
    )
    bk_np = np.ascontiguousarray(b_attn[c0k : c0k + 512].reshape(4, 128).T * ISQ)
    return {
        "x": np.ascontiguousarray(x[b]),
        "wk": wk_np,
        "wv": wv_np,
        "bk": bk_np,
        "a_neg": a_np,
        "b_o": b_np,
    }


def _core_wp(c, W_proj):
    h0 = HPC * (c % 2)
    r0 = h0 * HD
    return np.ascontiguousarray(
        W_proj[r0 : r0 + 512, :].reshape(4, 128, 1024).transpose(1, 0, 2)
    )


def _mask_mats():
    import ml_dtypes

    p = np.arange(128)
    a_np = np.where(p[:, None] <= p[None, :], np.float32(NEG), np.float32(0.0)).astype(
        ml_dtypes.bfloat16
    )
    il = np.arange(512)
    b_np = np.zeros((128, 4, 512), dtype=np.float32)
    for oi in range(4):
        b_np[:, oi, :] = (il[None, :] < (p[:, None] + 128 * oi)).astype(np.float32)
    return np.ascontiguousarray(a_np), np.ascontiguousarray(b_np.astype(ml_dtypes.bfloat16))


def kernel(x, W_attn, b_attn, W_proj, b_proj, **_unused):
    x = np.asarray(x, dtype=np.float32)
    W_attn = np.asarray(W_attn, dtype=np.float32)
    b_attn = np.asarray(b_attn, dtype=np.float32)
    W_proj = np.asarray(W_proj, dtype=np.float32)
    b_proj = np.asarray(b_proj, dtype=np.float32)

    nc = _get_program()
    a_np, b_np = _mask_mats()
    in_maps = []
    for c in range(NCORES):
        m = _core_inputs(c, x, W_attn, b_attn, a_np, b_np)
        m["wp"] = _core_wp(c, W_proj)
        in_maps.append(m)

    res = run_bass_kernel_spmd(nc, in_maps, core_ids=list(range(NCORES)))

    bias_row = b_proj + b_attn[2 * D : 3 * D] @ W_proj
    out = np.empty((B, T, D), dtype=np.float32)
    for b in range(B):
        out[b] = res.results[2 * b]["out"] + res.results[2 * b + 1]["out"] + bias_row
    return out



# revision 7
# speedup vs baseline: 2.4061x; 2.4061x over previous
"""Trainium2 Bass kernel for nn_CausalSelfAttention_38216619000057.

Reference semantics (faithful to the source bug q = k):
    qkv = x @ W_attn + b_attn ; _, k, v = split(qkv)
    S = (K K^T) * D**-0.5  (per head, causal-masked), P = softmax(S)
    out = (P V) reshaped @ W_proj + b_proj

Sharding over 8 cores: data-parallel on B (4), tensor-parallel on heads (2
groups of 8). Core c handles batch c//2, heads 8*(c%2)..8*(c%2)+7, and
produces a partial projection output; the host sums the two partials per
batch and adds b_proj + b_v @ W_proj (the V-bias contribution commutes
through softmax because rows of P sum to 1).

Precision / engine strategy (PE is the bottleneck):
  * The attention-logit path runs in fp8e4m3 with DoubleRow perf mode
    (2 moving rows/cycle): K projection, S^T = K K^T, and the causal-mask
    ramp matmuls. Softmax normalization + output averaging tame the fp8
    quantization noise (~0.5-1% final error, budget 2e-2).
  * Everything whose error lands directly in the output stays >= bf16:
    x^T/V-weights bf16, PV matmul bf16, output projection bf16, exp output
    bf16, PSUM f32.
  * x is transposed on the HOST and uploaded twice (fp8 pair-layout for the
    K matmul, bf16 for V) -- no on-device transposes at all.
  * Softmax denominators come free via a ones-column appended to V; the
    reciprocal row is broadcast across partitions on the otherwise-idle
    GPSIMD engine instead of a PE matmul.
  * Causal mask: fp8 DoubleRow matmul accumulates -224*(j-i)^+ (clamped
    ramp from triangular 0/1 factors) so exp() flushes masked entries to 0.

Work is software-pipelined per 512-query chunk: the K/V projection pieces of
chunk ci+1 and the output-projection pieces of chunk ci-1 are interleaved
between the attention head-pieces of chunk ci, keeping PE fed while the
Activation engine (exp, the #2 engine) drains its backlog.
"""

import threading

import numpy as np

import concourse.bacc as bacc
import concourse.mybir as mybir
import concourse.tile as tile
from concourse.bass_utils import run_bass_kernel_spmd

B, T, D = 4, 2048, 1024
H = 16
HD = 64
NCORES = 8
HPC = 8  # heads per core
ISQ = float(D**-0.5) ** 0.5  # K is pre-scaled by sqrt(D**-0.5)
WK_SCALE = 64.0  # keeps fp8 W_k columns in e4m3 normal range
F32 = mybir.dt.float32
F32R = mybir.dt.float32r
BF16 = mybir.dt.bfloat16
F8 = mybir.dt.float8e4
NEG = -224.0  # e4m3 max normal is 240
DR = mybir.MatmulPerfMode.DoubleRow

Ident = mybir.ActivationFunctionType.Identity
Exp = mybir.ActivationFunctionType.Exp
Mult = mybir.AluOpType.mult

_cache_lock = threading.Lock()
_cached_nc = {}


def _declare_io(nc, synth=False):
    kind = "Internal" if synth else "ExternalInput"
    ts = {}
    # x^T in fp8 pair layout for the DoubleRow K matmul:
    # x8[p, ei, eb, t] = x[t, 256*eb + 128*ei + p]
    ts["x8"] = nc.dram_tensor("x8", [128, 2, 4, T], F8, kind=kind)
    # x^T in bf16 for the V matmul: xb[p, eb, t] = x[t, 128*eb + p]
    ts["xb"] = nc.dram_tensor("xb", [128, 8, T], BF16, kind=kind)
    # W_k fp8 (x WK_SCALE), permuted so PSUM partitions land in kt8 layout:
    # wk[p, ei, eb, u, j] with u=(m,di), j=32a+d -> k-col (4m+a)*64+32*di+d
    ts["wk"] = nc.dram_tensor("wk", [128, 2, 4, 4, 128], F8, kind=kind)
    ts["wv"] = nc.dram_tensor("wv", [128, 8, 512], BF16, kind=kind)
    ts["wp"] = nc.dram_tensor("wp", [128, 4, 1024], BF16, kind=kind)
    ts["bk"] = nc.dram_tensor("bk", [128, 4], F32, kind=kind)
    ts["a_neg"] = nc.dram_tensor("a_neg", [64, 2, 128], F8, kind=kind)
    ts["b_o"] = nc.dram_tensor("b_o", [64, 2, 4, 512], F8, kind=kind)
    ts["out"] = nc.dram_tensor("out", [T, D], F32, kind="Internal" if synth else "ExternalOutput")
    if synth:
        ts["done"] = nc.dram_tensor("done", [1, 4], F32, kind="ExternalOutput")
    return ts


def _synth_init(nc, tc, io):
    """Fill the Internal input tensors with benign constants on device."""
    with tc.tile_pool(name="init", bufs=1) as pool:
        zt = pool.tile([128, 8192], F32, name="init_t")
        nc.vector.memset(zt[:], 0.0)
        nc.sync.dma_start(
            io["x8"][:],
            zt[:, 0:4096].bitcast(F8).rearrange("p (i e t) -> p i e t", i=2, e=4),
        )
        nc.sync.dma_start(
            io["xb"][:],
            zt[:, 0:8192].bitcast(BF16).rearrange("p (e t) -> p e t", e=8),
        )
        nc.sync.dma_start(
            io["wk"][:],
            zt[:, 0:1024].bitcast(F8).rearrange("p (i e u j) -> p i e u j", i=2, e=4, u=4),
        )
        nc.sync.dma_start(
            io["wv"][:],
            zt[:, 0:2048].bitcast(BF16).rearrange("p (e n) -> p e n", e=8),
        )
        nc.sync.dma_start(
            io["wp"][:],
            zt[:, 0:2048].bitcast(BF16).rearrange("p (h n) -> p h n", h=4),
        )
        nc.sync.dma_start(io["bk"][:], zt[:, 0:4])
        nc.sync.dma_start(
            io["a_neg"][:],
            zt[0:64, 0:64].bitcast(F8).rearrange("p (i j) -> p i j", i=2),
        )
        nc.sync.dma_start(
            io["b_o"][:],
            zt[0:64, 0:1024].bitcast(F8).rearrange("p (i o q) -> p i o q", i=2, o=4),
        )


def _emit_body(nc, tc, io, g):
    """One full forward pass. g holds the persistent SBUF tiles."""
    kt8, v_ones, o_t = g["kt8"], g["v_ones"], g["o_t"]
    kt8b = g["kt8b"]
    x8_sb, xb_sb = g["x8_sb"], g["xb_sb"]
    wk_sb, wv_sb, wp_sb = g["wk_sb"], g["wv_sb"], g["wp_sb"]
    an_sb, bo_sb, bk_sb = g["an_sb"], g["bo_sb"], g["bk_sb"]
    out = io["out"]

    with (
        tc.tile_pool(name="ps_s", bufs=2, space="PSUM") as ps_s,
        tc.tile_pool(name="ps_pv", bufs=2, space="PSUM") as ps_pv,
        tc.tile_pool(name="ps1", bufs=2, space="PSUM") as ps1,
        tc.tile_pool(name="ebuf", bufs=6) as ebuf,
        tc.tile_pool(name="rbuf", bufs=8) as rbuf,
        tc.tile_pool(name="obuf", bufs=3) as obuf,
    ):

        def a_pieces(ci):
            """K and V projection pieces for t-chunk ci (8 pieces)."""
            ps = []
            cs = slice(ci * 512, ci * 512 + 512)

            def k_piece(u, cs=cs, ci=ci):
                kps = ps1.tile([128, 512], F32, tag="ps1", name="kps")
                for eb in range(4):
                    nc.tensor.matmul(
                        kps[:],
                        wk_sb[:, :, eb, u, :],
                        x8_sb[:, :, eb, cs],
                        start=(eb == 0),
                        stop=(eb == 3),
                        perf_mode=DR,
                    )
                m, di = u // 2, u % 2
                nc.scalar.activation(
                    kt8[:, di, m, cs],
                    kps[:],
                    Ident,
                    bias=bk_sb[:, u : u + 1],
                    scale=ISQ / WK_SCALE,
                )

            def v_piece(tbl, ci=ci):
                vps = ps1.tile([128, 512], F32, tag="ps1", name="vps")
                tb = 4 * ci + tbl
                for eb in range(8):
                    nc.tensor.matmul(
                        vps[:],
                        xb_sb[:, eb, tb * 128 : (tb + 1) * 128],
                        wv_sb[:, eb, :],
                        start=(eb == 0),
                        stop=(eb == 7),
                    )
                nc.vector.tensor_copy(
                    v_ones[:, tb, :].rearrange("p (h c) -> p h c", c=65)[:, :, 0:64],
                    vps[:].rearrange("p (h c) -> p h c", c=64),
                )

            def kshift_piece(cs=cs):
                # matmul operands may only start at partition 0/32/64; shift
                # the upper head-groups down so S reads at bases {0, 32}
                nc.sync.dma_start(kt8b[:, :, :, cs], kt8[64:128, :, :, cs])

            for u in range(4):
                ps.append(lambda u=u: k_piece(u))
            ps.append(kshift_piece)
            for tbl in range(4):
                ps.append(lambda tbl=tbl: v_piece(tbl))
            return ps

        def b_head_piece(ci, l):
            """Attention for local head l (=4m+a) over query chunk ci."""
            m, a = l // 4, l % 4
            src = kt8 if a < 2 else kt8b
            pr = slice(32 * (a % 2), 32 * (a % 2) + 32)
            cs = slice(ci * 512, ci * 512 + 512)
            njb = 4 * ci + 4
            pv = ps_pv.tile([65, 512], F32, tag="pv", name="pv")
            for jbp in range(njb // 2):
                sps = ps_s.tile([128, 2, 512], F32, tag="s", name="sps")
                for half in (0, 1):
                    jb = 2 * jbp + half
                    crossing = jb >= 4 * ci
                    nc.tensor.matmul(
                        sps[:, half, :],
                        src[pr, :, m, jb * 128 : (jb + 1) * 128],
                        src[pr, :, m, cs],
                        start=True,
                        stop=not crossing,
                        perf_mode=DR,
                    )
                    if crossing:
                        oi = jb - 4 * ci
                        nc.tensor.matmul(
                            sps[:, half, :],
                            an_sb[:],
                            bo_sb[:, :, oi, :],
                            start=False,
                            stop=True,
                            perf_mode=DR,
                        )
                eps = ebuf.tile([128, 2, 512], BF16, tag="e", name="eps")
                nc.scalar.activation(eps[:], sps[:], Exp)
                for half in (0, 1):
                    jb = 2 * jbp + half
                    nc.tensor.matmul(
                        pv[:],
                        v_ones[:, jb, 65 * l : 65 * l + 65],
                        eps[:, half, :],
                        start=(jb == 0),
                        stop=(jb == njb - 1),
                    )
            r_row = rbuf.tile([1, 512], F32, tag="rr", name="r_row")
            nc.vector.reciprocal(r_row[:], pv[64:65, :])
            r_bc = rbuf.tile([64, 512], F32, tag="rb", name="r_bc")
            nc.gpsimd.partition_broadcast(r_bc[:], r_row[:])
            q, hp = l // 4, l % 4
            nc.vector.tensor_tensor(
                o_t[64 * q : 64 * q + 64, hp, cs], pv[0:64, :], r_bc[:], Mult
            )

        def proj_pieces(ci):
            """Output projection pieces for t-chunk ci (8 pieces)."""
            ps = []

            def p_piece(tbl, nch, ci=ci):
                tb = 4 * ci + tbl
                ops_ = ps1.tile([128, 512], F32, tag="ps1", name="ops")
                for hp2 in range(4):
                    nc.tensor.matmul(
                        ops_[:],
                        o_t[:, hp2, tb * 128 : (tb + 1) * 128],
                        wp_sb[:, hp2, nch * 512 : (nch + 1) * 512],
                        start=(hp2 == 0),
                        stop=(hp2 == 3),
                    )
                ob = obuf.tile([128, 512], F32, tag="ob", name="ob")
                nc.vector.tensor_copy(ob[:], ops_[:])
                nc.sync.dma_start(
                    out[tb * 128 : (tb + 1) * 128, nch * 512 : (nch + 1) * 512],
                    ob[:],
                )

            for tbl in range(4):
                for nch in range(2):
                    ps.append(lambda tbl=tbl, nch=nch: p_piece(tbl, nch))
            return ps

        for f in a_pieces(0):
            f()
        for ci in range(4):
            apcs = a_pieces(ci + 1) if ci < 3 else []
            ppcs = proj_pieces(ci - 1) if ci > 0 else []
            for i in range(8):
                b_head_piece(ci, i)
                if i < len(apcs):
                    apcs[i]()
                if i < len(ppcs):
                    ppcs[i]()
            for f in apcs[8:]:
                f()
            for f in ppcs[8:]:
                f()
        for f in proj_pieces(3):
            f()


def _build_program(nreps: int = 1, synth: bool = False):
    nc = bacc.Bacc("TRN2", target_bir_lowering=False)
    io = _declare_io(nc, synth=synth)

    with tile.TileContext(nc) as tc:
        if synth:
            _synth_init(nc, tc, io)
        with tc.tile_pool(name="singles", bufs=1) as singles:
            g = {}
            g["kt8"] = singles.tile([128, 2, 2, T], F8, name="kt8")
            g["kt8b"] = singles.tile([64, 2, 2, T], F8, name="kt8b")
            g["v_ones"] = singles.tile([128, 16, HPC * 65], BF16, name="v_ones")
            g["o_t"] = singles.tile([128, 4, T], BF16, name="o_t")
            g["x8_sb"] = singles.tile([128, 2, 4, T], F8, name="x8_sb")
            g["xb_sb"] = singles.tile([128, 8, T], BF16, name="xb_sb")
            g["wk_sb"] = singles.tile([128, 2, 4, 4, 128], F8, name="wk_sb")
            g["wv_sb"] = singles.tile([128, 8, 512], BF16, name="wv_sb")
            g["wp_sb"] = singles.tile([128, 4, 1024], BF16, name="wp_sb")
            g["an_sb"] = singles.tile([64, 2, 128], F8, name="an_sb")
            g["bo_sb"] = singles.tile([64, 2, 4, 512], F8, name="bo_sb")
            g["bk_sb"] = singles.tile([128, 4], F32, name="bk_sb")

            nc.sync.dma_start(g["wk_sb"][:], io["wk"][:])
            nc.sync.dma_start(g["wv_sb"][:], io["wv"][:])
            nc.sync.dma_start(g["wp_sb"][:], io["wp"][:])
            nc.sync.dma_start(g["an_sb"][:], io["a_neg"][:])
            nc.sync.dma_start(g["bo_sb"][:], io["b_o"][:])
            nc.sync.dma_start(g["bk_sb"][:], io["bk"][:])
            for ci in range(4):
                cs = slice(ci * 512, ci * 512 + 512)
                nc.sync.dma_start(g["x8_sb"][:, :, :, cs], io["x8"][:, :, :, cs])
                nc.sync.dma_start(g["xb_sb"][:, :, cs], io["xb"][:, :, cs])
            nc.vector.memset(
                g["v_ones"][:]
                .rearrange("p t (h c) -> p t h c", c=65)[:, :, :, 64:65],
                1.0,
            )

            for _rep in range(nreps):
                _emit_body(nc, tc, io, g)

            if synth:
                with tc.tile_pool(name="fin", bufs=1) as fin:
                    dn = fin.tile([1, 4], F32, name="dn")
                    nc.vector.memset(dn[:], 1.0)
                    nc.sync.dma_start(io["done"][:], dn[:])

    nc.compile()
    return nc


def _build_null_program():
    """Same I/O signature, trivial body -- for wall-clock differencing."""
    nc = bacc.Bacc("TRN2", target_bir_lowering=False)
    io = _declare_io(nc)
    with tile.TileContext(nc) as tc:
        with tc.tile_pool(name="sb", bufs=2) as sb:
            t = sb.tile([128, 512], F32)
            nc.sync.dma_start(t[:].bitcast(BF16), io["xb"][:, 0, 0:1024])
            for tb in range(16):
                for nch in range(2):
                    nc.sync.dma_start(
                        io["out"][
                            tb * 128 : (tb + 1) * 128, nch * 512 : (nch + 1) * 512
                        ],
                        t[:],
                    )
    nc.compile()
    return nc


def _get_program(nreps: int = 1, synth: bool = False):
    with _cache_lock:
        key = (nreps, synth)
        if key not in _cached_nc:
            _cached_nc[key] = _build_program(nreps, synth)
        return _cached_nc[key]


def _core_inputs(c, x, W_attn, b_attn, a_np, b_np):
    import ml_dtypes

    f8 = ml_dtypes.float8_e4m3
    bf16 = ml_dtypes.bfloat16
    b = c // 2
    h0 = HPC * (c % 2)
    c0k = D + h0 * HD
    c0v = 2 * D + h0 * HD
    xt = np.ascontiguousarray(x[b].T)  # [D, T]
    # x8[p, ei, eb, t] = xt[256*eb + 128*ei + p, t]
    x8 = np.ascontiguousarray(
        xt.reshape(4, 2, 128, T).transpose(2, 1, 0, 3).astype(f8)
    )
    # xb[p, eb, t] = xt[128*eb + p, t]
    xb = np.ascontiguousarray(xt.reshape(8, 128, T).transpose(1, 0, 2).astype(bf16))
    # wk[p, ei, eb, u, j=32a+d] = W[256eb+128ei+p, c0k + (4m+a)*64+32di+d]*WK_SCALE
    wkc = W_attn[:, c0k : c0k + 512] * WK_SCALE  # [1024, 512]
    # k-col layout: (m, a, di, d) -> col (4m+a)*64 + 32di + d
    wkc = wkc.reshape(4, 2, 128, 2, 4, 2, 32)  # [eb, ei, p, m, a, di, d]
    wk = np.ascontiguousarray(
        wkc.transpose(2, 1, 0, 3, 5, 4, 6)  # [p, ei, eb, m, di, a, d]
        .reshape(128, 2, 4, 4, 128)
        .astype(f8)
    )
    wv = np.ascontiguousarray(
        W_attn[:, c0v : c0v + 512].reshape(8, 128, 512).transpose(1, 0, 2).astype(bf16)
    )
    # bk[p=32a+d, u=(m,di)] = b_attn[c0k + (4m+a)*64 + 32di + d] * ISQ
    bkc = b_attn[c0k : c0k + 512].reshape(2, 4, 2, 32)  # [m, a, di, d]
    bk = np.ascontiguousarray(
        (bkc.transpose(1, 3, 0, 2) * ISQ)  # [a, d, m, di]
        .reshape(128, 4)
        .astype(np.float32)
    )
    return {
        "x8": x8,
        "xb": xb,
        "wk": wk,
        "wv": wv,
        "bk": bk,
        "a_neg": a_np,
        "b_o": b_np,
    }


def _core_wp(c, W_proj):
    import ml_dtypes

    h0 = HPC * (c % 2)
    r0 = h0 * HD
    # wp[p=64q+d, hp, n] = W_proj[r0 + (4q+hp)*64 + d, n]
    return np.ascontiguousarray(
        W_proj[r0 : r0 + 512, :]
        .reshape(2, 4, 64, 1024)
        .transpose(0, 2, 1, 3)
        .reshape(128, 4, 1024)
        .astype(ml_dtypes.bfloat16)
    )


def _mask_mats():
    import ml_dtypes

    f8 = ml_dtypes.float8_e4m3
    P = np.arange(128)
    an = np.where(P[:, None] <= P[None, :], np.float32(NEG), np.float32(0.0))
    il = np.arange(512)
    bo = np.zeros((128, 4, 512), dtype=np.float32)
    for oi in range(4):
        bo[:, oi, :] = (il[None, :] < (P[:, None] + 128 * oi)).astype(np.float32)
    # split contraction dim 128 -> [64, 2] for DoubleRow (P = 64*i2 + p')
    a_np = np.ascontiguousarray(an.reshape(2, 64, 128).transpose(1, 0, 2).astype(f8))
    b_np = np.ascontiguousarray(
        bo.reshape(2, 64, 4, 512).transpose(1, 0, 2, 3).astype(f8)
    )
    return a_np, b_np


def kernel(x, W_attn, b_attn, W_proj, b_proj, **_unused):
    x = np.asarray(x, dtype=np.float32)
    W_attn = np.asarray(W_attn, dtype=np.float32)
    b_attn = np.asarray(b_attn, dtype=np.float32)
    W_proj = np.asarray(W_proj, dtype=np.float32)
    b_proj = np.asarray(b_proj, dtype=np.float32)

    nc = _get_program()
    a_np, b_np = _mask_mats()
    in_maps = []
    for c in range(NCORES):
        m = _core_inputs(c, x, W_attn, b_attn, a_np, b_np)
        m["wp"] = _core_wp(c, W_proj)
        in_maps.append(m)

    res = run_bass_kernel_spmd(nc, in_maps, core_ids=list(range(NCORES)))

    bias_row = b_proj + b_attn[2 * D : 3 * D] @ W_proj
    out = np.empty((B, T, D), dtype=np.float32)
    for b in range(B):
        out[b] = res.results[2 * b]["out"] + res.results[2 * b + 1]["out"] + bias_row
    return out
